# revision 55
# baseline (speedup 1.0000x reference)
"""Trainium2 Bass kernel for nn_MultiHeadAttention_7834020348049.

Reference computation (per token, no cross-token interaction):
    qn  = LayerNorm(q) * gamma_m + beta_m
    kvn = LayerNorm(kv) * gamma_l + beta_l
    Q = qn @ Wq.T ; K,V = split(kvn @ Wkv.T)
    per token: scores[h,g] = Q[h,:] . K[g,:] / sqrt(128)  (8x8 over heads)
    ctx[h,:] = softmax_g(scores) @ V
    out = ctx @ Wo.T

Sharding: pure data-parallel over the 16*2048 = 32768 tokens -> 4096/core.

v3 pipeline (all-fp16 matmuls at 1 cycle/row, bf16 softmax for range):
  Same building blocks as v2 (token-major LN -> PE transpose ->
  weight-stationary projections -> masked per-token 8x8 attention ->
  feature-major ctx -> O-projection), but the schedule is restructured
  so the PE never starves (638.6us -> ~567us):
  - attention for chunk c-1 is interleaved INSIDE chunk c's projection
    phase (proj m-groups alternate with attention tile fronts/backs/
    O-projections). The softmax DVE/ACT chains get a full ~65us period
    of runway instead of a ~20us tail window, removing the per-chunk
    boundary stall and the per-chunk HAM re-throttle it caused (PE now
    holds K=8/8 for the whole run; ~94% PE busy).
  - QT/KT/VT double-buffered (chunk c writes set c%2 while attention
    reads set (c-1)%2); kv transposes of chunk c run early in period c
    itself (only the K/V m-groups need them), so the prologue critical
    path is just q(0) + Wq.
  - weight DMAs are single strided descriptors, column-sliced in
    first-use order (per-core HBM is ~341 GB/s and the prologue is
    DMA-bound).
  - PSUM: proj + O-proj share a 3-deep rotation, transposes 3-deep,
    scores 2 (8 banks total), so accumulation never waits on evacuation.
  - tail: last chunk's score/softmax chains pre-run inside its own
    projection period; only V-transposes/backs/O-projections trail.
  fp8 was evaluated and rejected: e4m3 DoubleRow would cut projection
  cycles 2x but costs ~2.6% RMS per quantized operand; simulated
  end-to-end rel_err 6.4e-2 (vs 2e-2 budget), and mixed-k-tile splits
  scale only as sqrt(fraction). This is an fp16-floor kernel.
"""
import sys, os
sys.path.insert(0, "/opt/trn_rl_repo")
os.environ.setdefault("JAX_PLATFORMS", "cpu")

from contextlib import ExitStack
import numpy as np
import ml_dtypes

import concourse.bass as bass
import concourse.bacc as bacc
import concourse.tile as tile
from concourse import mybir
from concourse.masks import make_identity
from concourse.bass_utils import run_bass_kernel_spmd

F32 = mybir.dt.float32
F16 = mybir.dt.float16
BF16 = mybir.dt.bfloat16

DIM = 1024
HEADS = 8
DHEAD = 128
NCORES = 8

TC = 512   # tokens per chunk (projection moving-dim)
TT = 128   # tokens per tile (partition dim)
TS = 16    # tokens per attention sub-tile
KT_F = DIM // 128  # 8 k-tiles for the 1024-feature contraction


def head_windows(t, h, nwin):
    """head-h columns across nwin h-major sub-tile windows of an
    interleaved [128, nwin*128] tensor: window w holds cols
    w*128 + h*16 + t_local. Two free dims, 16-elem packed runs."""
    return bass.AP(tensor=t.tensor, offset=t.offset + h * TS,
                   ap=[t.ap[0], [128, nwin], [1, TS]])


def build_nc(T, with_bias_q=False, with_bias_kv=False):
    nc = bacc.Bacc(trn_type="TRN2", target_bir_lowering=False)

    q_d = nc.dram_tensor("q", [T, DIM], F16, kind="ExternalInput").ap()
    kv_d = nc.dram_tensor("kv", [T, DIM], F16, kind="ExternalInput").ap()
    wq_d = nc.dram_tensor("wq", [DIM, DIM], F16, kind="ExternalInput").ap()
    wkv_d = nc.dram_tensor("wkv", [DIM, 2 * DIM], F16, kind="ExternalInput").ap()
    wo_d = nc.dram_tensor("wo", [DIM, DIM], F16, kind="ExternalInput").ap()
    mask_d = nc.dram_tensor("mask", [TT, 4 * TT], BF16, kind="ExternalInput").ap()
    bq_d = bkv_d = None
    if with_bias_q:
        bq_d = nc.dram_tensor("bq", [1, DIM], F16, kind="ExternalInput").ap()
    if with_bias_kv:
        bkv_d = nc.dram_tensor("bkv", [1, 2 * DIM], F16, kind="ExternalInput").ap()
    out_d = nc.dram_tensor("out", [T, DIM], F16, kind="ExternalOutput").ap()

    NCH = T // TC        # chunks
    TPC = TC // TT       # tiles per chunk (4)
    SPT = TT // TS       # sub-tiles per tile (8)

    with tile.TileContext(nc) as tc, ExitStack() as ctx:
        # ---------------- static SBUF ----------------
        singles = ctx.enter_context(tc.tile_pool(name="singles", bufs=1))
        ident = singles.tile([128, 128], F16)
        make_identity(nc, ident[:])
        mask = singles.tile([TT, 4 * TT], BF16)
        eps = singles.tile([128, 1], F32)
        nc.vector.memset(eps[:], 1e-5)

        wq_sb = singles.tile([128, KT_F, DIM], F16)
        wkv_sb = singles.tile([128, KT_F, 2 * DIM], F16)
        wo_sb = singles.tile([128, KT_F, DIM], F16)

        if with_bias_q or with_bias_kv:
            ones_row = singles.tile([1, TC], F16)
            nc.vector.memset(ones_row[:], 1.0)
        if with_bias_q:
            bq_sb = singles.tile([1, DIM], F16)
        if with_bias_kv:
            bkv_sb = singles.tile([1, 2 * DIM], F16)

        # feature-major activations; double-buffered at both stages
        qkv_p = ctx.enter_context(tc.tile_pool(name="qkvT", bufs=2))
        qkt_p = ctx.enter_context(tc.tile_pool(name="qkt", bufs=2))

        raw_p = ctx.enter_context(tc.tile_pool(name="raw", bufs=16))
        st_p = ctx.enter_context(tc.tile_pool(name="stats", bufs=4))
        e_p = ctx.enter_context(tc.tile_pool(name="ebuf", bufs=2))
        em_p = ctx.enter_context(tc.tile_pool(name="embuf", bufs=2))
        p_p = ctx.enter_context(tc.tile_pool(name="pbuf", bufs=2))
        z_p = ctx.enter_context(tc.tile_pool(name="zbuf", bufs=4))
        l_p = ctx.enter_context(tc.tile_pool(name="lbuf", bufs=2))
        vb_p = ctx.enter_context(tc.tile_pool(name="vbuf", bufs=3))
        ctxT_p = ctx.enter_context(tc.tile_pool(name="ctxT", bufs=2))
        outsb_p = ctx.enter_context(tc.tile_pool(name="outsb", bufs=2))

        ps_tr = ctx.enter_context(tc.tile_pool(name="ps_tr", bufs=3, space="PSUM"))
        ps_mm = ctx.enter_context(tc.tile_pool(name="ps_mm", bufs=3, space="PSUM"))
        ps_s = ctx.enter_context(tc.tile_pool(name="ps_s", bufs=2, space="PSUM"))
        ps_o = ps_mm  # proj and O-proj share one PSUM rotation

        # per-chunk raw tiles, alive from DMA (period c-1) to transpose
        # (end of period c)
        raw_tiles = {}

        def emit_loads(c, names=("q", "kv")):
            if c >= NCH:
                return
            for it in range(TPC):
                tok0 = c * TC + it * TT
                for name, src in (("q", q_d), ("kv", kv_d)):
                    if name not in names:
                        continue
                    x = raw_p.tile([128, DIM], F16, tag="raw")
                    nc.sync.dma_start(x[:], src[tok0:tok0 + TT, :])
                    raw_tiles[(c, it, name)] = x

        def emit_ln(c, it, names=("q", "kv"), apply_act=False):
            """LayerNorm tensors of tile (c, it) in place. apply_act moves
            the normalize-apply to ScalarE (prologue: shortens the serial
            DVE chain gating the first transposes)."""
            if c >= NCH:
                return
            for name in names:
                x = raw_tiles[(c, it, name)]
                stats = st_p.tile([128, 2, 6], F32, tag="bn")
                xg = x.rearrange("p (n f) -> p n f", n=2)
                for i in range(2):
                    nc.vector.bn_stats(out=stats[:, i, :], in_=xg[:, i, :])
                mv = st_p.tile([128, 2], F32, tag="mv")
                nc.vector.bn_aggr(out=mv[:], in_=stats[:])
                rstd = st_p.tile([128, 1], F32, tag="rstd")
                nc.scalar.activation(out=rstd[:], in_=mv[:, 1:2],
                                     func=mybir.ActivationFunctionType.Sqrt,
                                     bias=eps[:], scale=1.0)
                nc.vector.reciprocal(out=rstd[:], in_=rstd[:])
                if apply_act:
                    nmr = st_p.tile([128, 1], F32, tag="rstd")
                    nc.vector.tensor_scalar(out=nmr[:], in0=mv[:, 0:1],
                                            scalar1=rstd[:], scalar2=-1.0,
                                            op0=mybir.AluOpType.mult,
                                            op1=mybir.AluOpType.mult)
                    nc.scalar.activation(
                        out=x[:], in_=x[:],
                        func=mybir.ActivationFunctionType.Identity,
                        bias=nmr[:], scale=rstd[:])
                else:
                    nc.vector.tensor_scalar(out=x[:], in0=x[:],
                                            scalar1=mv[:, 0:1],
                                            scalar2=rstd[:],
                                            op0=mybir.AluOpType.subtract,
                                            op1=mybir.AluOpType.mult)

        chunk_qkv = {}     # c -> (qnT, kvnT) feature-major LN'd activations
        chunk_qkt = {}     # c -> (QT, KT, VT) projected per-head windows

        def alloc_qkv(c):
            if c >= NCH:
                return
            qnT = qkv_p.tile([128, KT_F, TC], F16, tag="qnT")
            kvnT = qkv_p.tile([128, KT_F, TC], F16, tag="kvnT")
            chunk_qkv[c] = (qnT, kvnT)

        def alloc_qkt(c):
            QT = qkt_p.tile([128, TC * HEADS], F16, tag="QT")
            KT = qkt_p.tile([128, TC * HEADS], F16, tag="KT")
            VT = qkt_p.tile([128, TC * HEADS], F16, tag="VT")
            chunk_qkt[c] = (QT, KT, VT)

        def emit_trans_tile(c, it, names=("q", "kv")):
            """PE transpose LN'd tile (c, it) to feature-major qnT/kvnT."""
            if c >= NCH:
                return
            qnT, kvnT = chunk_qkv[c]
            for name, dstT in (("q", qnT), ("kv", kvnT)):
                if name not in names:
                    continue
                x = raw_tiles.pop((c, it, name))
                tp = ps_tr.tile([128, KT_F, 128], F16, tag="tr")
                for f in range(KT_F):
                    nc.tensor.transpose(
                        tp[:, f, :], x[:, f * 128:(f + 1) * 128],
                        ident[:], )
                nc.vector.tensor_copy(
                    out=dstT[:, :, it * TT:(it + 1) * TT], in_=tp[:])

        def emit_proj_group(c, j, tile_split=False):
            """One projection m-group: j in 0..7 -> Q head j;
            j in 8..23 -> KV output block j-8 (K heads then V heads).
            tile_split: emit per-token-tile N=128 matmuls so the group can
            start before all four input-tile transposes land (prologue)."""
            qnT, kvnT = chunk_qkv[c]
            QT, KT, VT = chunk_qkt[c]
            if j < HEADS:
                m = j
                ps = ps_mm.tile([128, TC], F32, tag="mm")
                if tile_split:
                    for it in range(TPC):
                        for k in range(KT_F):
                            nc.tensor.matmul(
                                ps[:, it * TT:(it + 1) * TT],
                                wq_sb[:, k, m * 128:(m + 1) * 128],
                                qnT[:, k, it * TT:(it + 1) * TT],
                                start=(k == 0),
                                stop=(k == KT_F - 1 and not with_bias_q),
                                skip_group_check=True)
                else:
                    for k in range(KT_F):
                        nc.tensor.matmul(
                            ps[:], wq_sb[:, k, m * 128:(m + 1) * 128],
                            qnT[:, k, :], start=(k == 0),
                            stop=(k == KT_F - 1 and not with_bias_q))
                if with_bias_q:
                    nc.tensor.matmul(
                        ps[:], bq_sb[:, m * 128:(m + 1) * 128],
                        ones_row[:], start=False, stop=True)
                nc.scalar.copy(out=head_windows(QT, m, TC // TS), in_=ps[:])
            else:
                m = j - HEADS
                ps = ps_mm.tile([128, TC], F32, tag="mm")
                for k in range(KT_F):
                    nc.tensor.matmul(
                        ps[:], wkv_sb[:, k, m * 128:(m + 1) * 128],
                        kvnT[:, k, :], start=(k == 0),
                        stop=(k == KT_F - 1 and not with_bias_kv))
                if with_bias_kv:
                    nc.tensor.matmul(
                        ps[:], bkv_sb[:, m * 128:(m + 1) * 128],
                        ones_row[:], start=False, stop=True)
                dst = KT if m < HEADS else VT
                nc.scalar.copy(out=head_windows(dst, m % HEADS, TC // TS),
                               in_=ps[:])

        def emit_attn_scores(c, it):
            """Scores and the softmax chain up to P (no V transposes)."""
            QT, KT, VT = chunk_qkt[c]
            t0 = it * TT
            E = e_p.tile([128, SPT, 128], BF16, tag="e")
            EM = em_p.tile([128, SPT, 128], BF16, tag="em")
            P = p_p.tile([128, SPT, 128], F16, tag="p")
            z = z_p.tile([128, SPT], F32, tag="z")
            zr = z_p.tile([128, SPT], F32, tag="zr")
            st = {"E": E, "EM": EM, "P": P, "z": z, "zr": zr, "t0": t0}
            spss = []
            for b in range(2):
                sps = ps_s.tile([128, 4, 128], F32, tag="s")
                for s4 in range(4):
                    c0 = (t0 + (b * 4 + s4) * TS) * HEADS
                    nc.tensor.matmul(
                        sps[:, s4, :],
                        QT[:, c0:c0 + 128], KT[:, c0:c0 + 128],
                        start=True, stop=True, skip_group_check=True)
                spss.append(sps)
            for b in range(2):
                nc.scalar.activation(
                    out=E[:, b * 4:(b + 1) * 4, :], in_=spss[b],
                    func=mybir.ActivationFunctionType.Exp, scale=1.0)
                nc.vector.tensor_tensor(
                    out=EM[:, b * 4:(b + 1) * 4, :],
                    in0=E[:, b * 4:(b + 1) * 4, :], in1=mask[:],
                    op=mybir.AluOpType.mult)
                nc.vector.tensor_reduce(
                    out=z[:, b * 4:(b + 1) * 4],
                    in_=EM[:, b * 4:(b + 1) * 4, :],
                    op=mybir.AluOpType.add, axis=mybir.AxisListType.X)
                nc.vector.reciprocal(out=zr[:, b * 4:(b + 1) * 4],
                                     in_=z[:, b * 4:(b + 1) * 4])
                for s4 in range(4):
                    s = b * 4 + s4
                    if s4 % 2 == 0:
                        nc.vector.tensor_scalar(
                            out=P[:, s, :], in0=EM[:, s, :],
                            scalar1=zr[:, s:s + 1], scalar2=None,
                            op0=mybir.AluOpType.mult)
                    else:
                        nc.scalar.activation(
                            out=P[:, s, :], in_=EM[:, s, :],
                            func=mybir.ActivationFunctionType.Copy,
                            scale=zr[:, s:s + 1])
            return st

        def emit_attn_vtr(c, it, st):
            """V transposes for tile (c, it) into st['Vb']."""
            QT, KT, VT = chunk_qkt[c]
            t0 = it * TT
            Vb = vb_p.tile([128, SPT, 128], F16, tag="vb")
            st["Vb"] = Vb
            vtp = ps_tr.tile([128, SPT, 128], F16, tag="tr")
            for s in range(SPT):
                c0 = (t0 + s * TS) * HEADS
                nc.tensor.transpose(
                    vtp[:, s, :], VT[:, c0:c0 + 128], ident[:])
            nc.scalar.copy(out=Vb[:], in_=vtp[:])

        def emit_attn_front(c, it):
            st = emit_attn_scores(c, it)
            emit_attn_vtr(c, it, st)
            return st

        def emit_attn_back(c, it, st):
            """P transposes, ctx matmuls, ctx copies; returns ctxT."""
            P, Vb = st["P"], st["Vb"]
            L = l_p.tile([128, SPT, 128], F16, tag="l")
            ctxT = ctxT_p.tile([128, HEADS, TT], F16, tag="ctxT")
            ptp = ps_tr.tile([128, SPT, 128], F16, tag="tr")
            for s in range(SPT):
                nc.tensor.transpose(ptp[:, s, :], P[:, s, :], ident[:])
            nc.vector.tensor_copy(out=L[:], in_=ptp[:])
            for b in range(2):
                cps = ps_s.tile([128, 4, 128], F32, tag="s")
                for s4 in range(4):
                    s = b * 4 + s4
                    nc.tensor.matmul(
                        cps[:, s4, :], Vb[:, s, :], L[:, s, :],
                        start=True, stop=True, skip_group_check=True)
                # batched reorder copy: src cols (h, s4, t) ->
                # ctxT[d][h][b*64 + s4*16 + t]
                src = bass.AP(tensor=cps.tensor, offset=cps.offset,
                              ap=[cps.ap[0], [TS, HEADS], [128, 4], [1, TS]])
                dst = bass.AP(tensor=ctxT.tensor,
                              offset=ctxT.offset + b * 64,
                              ap=[ctxT.ap[0], [TT, HEADS], [TS, 4], [1, TS]])
                if b == 0:
                    nc.scalar.copy(out=dst, in_=src)
                else:
                    nc.vector.tensor_copy(out=dst, in_=src)
            return ctxT

        def emit_oproj(c, it, ctxT, oh):
            tok0 = c * TC + it * TT
            pso = ps_o.tile([128, 512], F32, tag="mm")
            for h in range(HEADS):
                nc.tensor.matmul(
                    pso[:], ctxT[:, h, :],
                    wo_sb[:, h, oh * 512:(oh + 1) * 512],
                    start=(h == 0), stop=(h == HEADS - 1))
            osb = outsb_p.tile([128, 512], F16, tag="osb")
            nc.scalar.copy(out=osb[:], in_=pso[:])
            nc.sync.dma_start(
                out_d[tok0:tok0 + TT, oh * 512:(oh + 1) * 512], osb[:])

        # ---------------- prologue ----------------
        # DMA order matters: q(0) tiles and Wq gate the first projection.
        def w_slice(dst, src, width, c0, c1):
            """Load weight columns [c0:c1) (all KT_F k-tiles) in one DMA."""
            src_ap = bass.AP(tensor=src.tensor, offset=c0,
                             ap=[[width, 128], [128 * width, KT_F],
                                 [1, c1 - c0]])
            nc.sync.dma_start(dst[:, :, c0:c1], src_ap)

        # DMA order tracks first use: q(0) + Wq[heads 0-1] gate the first
        # projection m-groups; later weight column-slices stream in while
        # the first groups run.
        emit_loads(0, names=("q",))
        w_slice(wq_sb, wq_d, DIM, 0, 256)
        w_slice(wq_sb, wq_d, DIM, 256, DIM)
        emit_loads(0, names=("kv",))
        w_slice(wkv_sb, wkv_d, 2 * DIM, 0, DIM)
        emit_loads(1, names=("q",))
        w_slice(wkv_sb, wkv_d, 2 * DIM, DIM, 2 * DIM)
        emit_loads(1, names=("kv",))
        w_slice(wo_sb, wo_d, DIM, 0, DIM)
        nc.sync.dma_start(mask[:], mask_d)
        # chunk 0 kv transposes happen inside period 0 (they only gate the
        # K-projection groups); the prologue critical path is q(0) + Wq.

        # HAM warm-up: the PE is otherwise idle for ~8us while q(0)/Wq
        # stream in, so its first real work would run at the cold 1.2 GHz
        # clock. Dependency-free ident@ident matmuls fill the wait and
        # un-throttle the clock gate (normal-mode only: transpose-mode
        # doesn't register as PE-busy for HAM).
        warm = ps_mm.tile([128, 128], F32, tag="mm")
        for _ in range(112):
            nc.tensor.matmul(warm[:], ident[:], ident[:],
                             start=True, stop=True, skip_group_check=True)
        if with_bias_q:
            nc.sync.dma_start(bq_sb[:], bq_d)
        if with_bias_kv:
            nc.sync.dma_start(bkv_sb[:], bkv_d)

        alloc_qkv(0)
        for it in range(TPC):
            emit_ln(0, it, names=("q",))
            emit_trans_tile(0, it, names=("q",))
        for it in range(TPC):
            emit_ln(0, it, names=("kv",))

        # ---------------- main periods ----------------
        # period p: projections of chunk p, attention of chunk p-1,
        # LN+transpose of chunk p+1, DMA loads of chunk p+2.
        fronts = {}
        ctxs = {}
        last_sc = {}

        def F(p, it):
            def f():
                fronts[it] = emit_attn_front(p - 1, it)
            return f

        def B(p, it):
            def f():
                ctxs[it] = emit_attn_back(p - 1, it, fronts.pop(it))
            return f

        def O(p, it, oh):
            def f():
                emit_oproj(p - 1, it, ctxs[it], oh)
                if oh == 1:
                    ctxs.pop(it)
            return f

        def PG(p, j):
            return lambda: emit_proj_group(p, j)

        def LN1(p, it, name):
            return lambda: emit_ln(p + 1, it, names=(name,))

        def TRKV(p, it):
            return lambda: emit_trans_tile(p, it, names=("kv",))

        def TR1(p, it, name):
            return lambda: emit_trans_tile(p + 1, it, names=(name,))

        for p in range(NCH):
            alloc_qkv(p + 1)
            alloc_qkt(p)
            has_a = p >= 1
            last = p == NCH - 1
            seq = []
            A = seq.append
            # --- Q projections + kv transposes of THIS chunk + fronts ---
            A(PG(p, 0))
            if has_a: A(F(p, 0))
            A(PG(p, 1))
            A(PG(p, 2))
            if has_a: A(F(p, 1))
            A(PG(p, 3))
            A(PG(p, 4)); A(TRKV(p, 0))
            if has_a: A(B(p, 0))
            A(PG(p, 5)); A(TRKV(p, 1))
            if has_a: A(F(p, 2))
            A(PG(p, 6)); A(TRKV(p, 2))
            if has_a: A(O(p, 0, 0))
            A(PG(p, 7)); A(TRKV(p, 3))
            A(lambda: emit_loads(p + 2))
            # --- K projections + backs/oprojs ---
            A(PG(p, 8)); A(LN1(p, 0, "q"))
            if has_a: A(B(p, 1))
            A(PG(p, 9)); A(LN1(p, 1, "q"))
            if has_a: A(F(p, 3))
            A(PG(p, 10))
            if has_a: A(O(p, 0, 1))
            A(PG(p, 11)); A(LN1(p, 2, "q"))
            A(PG(p, 12)); A(LN1(p, 3, "q"))
            if has_a: A(B(p, 2))
            A(PG(p, 13))
            if has_a: A(O(p, 1, 0))
            A(PG(p, 14)); A(TR1(p, 0, "q"))
            A(PG(p, 15)); A(LN1(p, 0, "kv"))
            # --- V projections + tail of attention + transposes ---
            A(PG(p, 16))
            if has_a: A(B(p, 3))
            A(PG(p, 17)); A(TR1(p, 1, "q")); A(LN1(p, 1, "kv"))
            if last: A(lambda: last_sc.__setitem__(0, emit_attn_scores(p, 0)))
            A(PG(p, 18))
            if has_a: A(O(p, 1, 1))
            A(PG(p, 19)); A(LN1(p, 2, "kv"))
            A(PG(p, 20))
            if has_a: A(O(p, 2, 0))
            if last: A(lambda: last_sc.__setitem__(1, emit_attn_scores(p, 1)))
            A(PG(p, 21)); A(TR1(p, 2, "q")); A(LN1(p, 3, "kv"))
            A(PG(p, 22))
            if has_a: A(O(p, 2, 1))
            A(PG(p, 23)); A(TR1(p, 3, "q"))
            if has_a:
                A(O(p, 3, 0)); A(O(p, 3, 1))
            for f in seq:
                f()
            chunk_qkv.pop(p)
            if p >= 1:
                chunk_qkt.pop(p - 1)

        # ---------------- tail: attention of the last chunk ----------------
        # score chains for tiles 0/1 were pre-run inside period NCH-1;
        # here: V transposes, tiles 2/3 scores, backs and O-projections,
        # interleaved so the PE stays dense.
        lc = NCH - 1
        emit_attn_vtr(lc, 0, last_sc[0])
        emit_attn_vtr(lc, 1, last_sc[1])
        ctxs[0] = emit_attn_back(lc, 0, last_sc.pop(0))
        last_sc[2] = emit_attn_scores(lc, 2)
        emit_oproj(lc, 0, ctxs[0], 0)
        ctxs[1] = emit_attn_back(lc, 1, last_sc.pop(1))
        emit_attn_vtr(lc, 2, last_sc[2])
        emit_oproj(lc, 0, ctxs[0], 1)
        last_sc[3] = emit_attn_scores(lc, 3)
        emit_oproj(lc, 1, ctxs[1], 0)
        ctxs[2] = emit_attn_back(lc, 2, last_sc.pop(2))
        emit_attn_vtr(lc, 3, last_sc[3])
        emit_oproj(lc, 1, ctxs[1], 1)
        ctxs[3] = emit_attn_back(lc, 3, last_sc.pop(3))
        emit_oproj(lc, 2, ctxs[2], 0)
        emit_oproj(lc, 2, ctxs[2], 1)
        emit_oproj(lc, 3, ctxs[3], 0)
        emit_oproj(lc, 3, ctxs[3], 1)

    nc.finalize()
    return nc


def _host_mask():
    # h-major windows: row p = h*16+t, col q = g*16+t'; valid iff t == t'.
    # Tiled 4x horizontally for batched 4-subtile multiplies.
    m = np.zeros((TT, TT), np.float32)
    p = np.arange(TT)
    m[p[:, None] % TS == p[None, :] % TS] = 1.0
    return np.tile(m, (1, 4)).astype(ml_dtypes.bfloat16)


def kernel(q, kv, gamma_m, beta_m, gamma_l, beta_l, Wq, Wkv, Wo):
    q = np.asarray(q, np.float32)
    kv = np.asarray(kv, np.float32)
    bs, patch, _ = q.shape
    T_total = bs * patch
    T_core = T_total // NCORES

    scale = DHEAD ** (-0.5)
    # fold LN gamma into the projection weights, beta into bias vectors
    wq_eff = (np.asarray(Wq, np.float32) * np.asarray(gamma_m, np.float32)[None, :]) * scale
    bq = (np.asarray(Wq, np.float32) @ np.asarray(beta_m, np.float32)) * scale
    wkv_eff = np.asarray(Wkv, np.float32) * np.asarray(gamma_l, np.float32)[None, :]
    bkv = np.asarray(Wkv, np.float32) @ np.asarray(beta_l, np.float32)
    with_bias_q = bool(np.any(bq != 0.0))
    with_bias_kv = bool(np.any(bkv != 0.0))

    # kernel weight layout: [in, out], fp16
    wq_t = np.ascontiguousarray(wq_eff.T).astype(np.float16)
    wkv_t = np.ascontiguousarray(wkv_eff.T).astype(np.float16)
    wo_t = np.ascontiguousarray(np.asarray(Wo, np.float32).T).astype(np.float16)
    mask = _host_mask()

    nc = build_nc(T_core, with_bias_q, with_bias_kv)

    qf = q.reshape(T_total, DIM).astype(np.float16)
    kvf = kv.reshape(T_total, DIM).astype(np.float16)
    in_maps = []
    for i in range(NCORES):
        m = {
            "q": np.ascontiguousarray(qf[i * T_core:(i + 1) * T_core]),
            "kv": np.ascontiguousarray(kvf[i * T_core:(i + 1) * T_core]),
            "wq": wq_t, "wkv": wkv_t, "wo": wo_t, "mask": mask,
        }
        if with_bias_q:
            m["bq"] = bq.reshape(1, DIM).astype(np.float16)
        if with_bias_kv:
            m["bkv"] = bkv.reshape(1, 2 * DIM).astype(np.float16)
        in_maps.append(m)

    res = run_bass_kernel_spmd(nc, in_maps, list(range(NCORES)))
    global LAST_RESULTS
    LAST_RESULTS = res
    out = np.concatenate(
        [np.asarray(res.results[i]["out"], np.float32) for i in range(NCORES)],
        axis=0)
    return out.reshape(bs, patch, DIM)


LAST_RESULTS = None
